# revision 28
# baseline (speedup 1.0000x reference)
"""Attention-pooling Trainium2 kernel, v5.

Per-core (8 cores = 4 batches x 2 query-row halves): [2048, 4096] score
block via fp8 DoubleRow matmuls (256-contraction in one pass, 0.5
cyc/row). exp splits between ACT (exact exp, bf16 out, fused rowsum
accumulator) and DVE (Schraudolph uint16 bit-trick writing bf16 bit
patterns; GPSIMD can read neither PSUM nor do free-axis reduces, so it
only issues DMAs). Rowsums for DVE chunks ride a 4x-mode bf16
tensor_scalar pass with fused accumulator. Colsums are M=8
sliding-window matmuls: lhsT = [128,16] zero tile with recb in col 8;
key-chunk m uses cols [8-m:16-m] so its output lands on psum partition
m, all 128 matmuls accumulating into one PSUM bank.

Host: Q/K projections + fp8 quantization in, (w @ x) @ Wv finish out.
"""

import numpy as np

import concourse.bass as bass  # noqa: F401
import concourse.mybir as mybir
import concourse.tile as tile
from concourse import bacc

B, S, E = 4, 4096, 256
HALF = S // 2
P = 128
N_CORES = 8
QTILES = HALF // P     # 16
F32 = mybir.dt.float32
FP8 = mybir.dt.float8e4
BF16 = mybir.dt.bfloat16
U16 = mybir.dt.uint16
ALU = mybir.AluOpType

EXPW = 1024            # psum chunk width
NEXP = S // EXPW       # 4 chunks per q-tile

LOG2E = float(np.log2(np.e))
SCH_A = 128.0 * LOG2E / 16.0          # Schraudolph scale (raw scores -> bf16 bits)
SCH_B = 16256.0 - 8.0                 # 127<<7, -8 = sawtooth calibration

# per-(tile, chunk) exp engine: A=ACT exact exp, V=DVE Schraudolph
SCHED = ["VVAA", "VAAA", "VVAA", "VAAA", "VVAA", "VAAA", "VVAA", "VAAA",
         "VVAA", "VAAA", "VVAA", "VAAA", "VAAA", "VVAA", "VAAA", "VAAA"]
COLSUM_LAG = 2  # tiles between exp emission and tile colsum emission
NLT = 6         # recb lhsT ring depth (must exceed COLSUM_LAG + 1)


def _runs(pat):
    """Contiguous 'V' chunk runs [(start, len)] for one tile pattern."""
    out = []
    c = 0
    while c < NEXP:
        if pat[c] == "A":
            c += 1
            continue
        c0 = c
        while c < NEXP and pat[c] != "A":
            c += 1
        out.append((c0, c - c0))
    return out


def _emit(ctx, tc):
    nc = tc.nc

    qt_d = nc.dram_tensor("qt8", [P, 2, HALF], FP8, kind="ExternalInput")
    kt_d = nc.dram_tensor("kt8", [P, 2, S], FP8, kind="ExternalInput")
    w_d = nc.dram_tensor("w", [8, 1024], F32, kind="ExternalOutput")

    const = ctx.enter_context(tc.tile_pool(name="const", bufs=1))
    epool = ctx.enter_context(tc.tile_pool(name="epool", bufs=5))
    rsp = ctx.enter_context(tc.tile_pool(name="rsp", bufs=4))
    jkp = ctx.enter_context(tc.tile_pool(name="jkp", bufs=2))
    pp = ctx.enter_context(tc.tile_pool(name="pp", bufs=3, space="PSUM"))
    wp = ctx.enter_context(tc.tile_pool(name="wp", bufs=1, space="PSUM"))

    qt_sb = const.tile([P, 2, HALF], FP8, name="qt_sb")
    kt_sb = const.tile([P, 2, S], FP8, name="kt_sb")
    lt = [const.tile([P, 16], BF16, name=f"lt{i}") for i in range(NLT)]
    for i in range(NLT):
        nc.vector.memset(lt[i], 0.0)
    w_sb = const.tile([8, 1024], F32, name="w_sb")

    # ---- input DMAs: tiny first-needed slices, then bulk on 3 queues
    nc.scalar.dma_start(out=kt_sb[:, :, 0:512], in_=kt_d[:, :, 0:512])
    nc.sync.dma_start(out=qt_sb[:, :, 0:128], in_=qt_d[:, :, 0:128])
    nc.gpsimd.dma_start(out=kt_sb[:, :, 512:1024], in_=kt_d[:, :, 512:1024])
    nc.sync.dma_start(out=kt_sb[:, :, 1024:1536], in_=kt_d[:, :, 1024:1536])
    nc.scalar.dma_start(out=kt_sb[:, :, 1536:2048], in_=kt_d[:, :, 1536:2048])
    nc.gpsimd.dma_start(out=kt_sb[:, :, 2048:2560], in_=kt_d[:, :, 2048:2560])
    nc.sync.dma_start(out=kt_sb[:, :, 2560:3072], in_=kt_d[:, :, 2560:3072])
    nc.scalar.dma_start(out=kt_sb[:, :, 3072:4096], in_=kt_d[:, :, 3072:4096])
    nc.sync.dma_start(out=qt_sb[:, :, 128:1024], in_=qt_d[:, :, 128:1024])
    nc.scalar.dma_start(out=qt_sb[:, :, 1024:2048], in_=qt_d[:, :, 1024:2048])

    w_ps = [wp.tile([8, 512], F32, name=f"w_ps{r}") for r in range(2)]
    es_tiles = {}
    lt_of = {}
    pending = []  # deferred colsum matmul thunks, drained ~1 per chunk

    def drain_pending(n):
        for _ in range(min(n, len(pending))):
            pending.pop(0)()

    def emit_tile(qi):
        Es = epool.tile([P, S], BF16, tag="E", name=f"E{qi}")
        es_tiles[qi] = Es
        lt_of[qi] = lt[qi % NLT]
        pat = SCHED[qi]
        rs4 = rsp.tile([P, 4], F32, tag="rs4", name=f"rs4_{qi}")
        slot = 0
        for c in range(NEXP):
            ps = pp.tile([P, EXPW], F32, tag="ps", name=f"ps{qi}_{c}")
            for h in range(2):
                t0 = c * EXPW + h * 512
                nc.tensor.matmul(
                    ps[:, h * 512:(h + 1) * 512],
                    qt_sb[:, :, qi * P:(qi + 1) * P],
                    kt_sb[:, :, t0:t0 + 512],
                    start=True, stop=True,
                    perf_mode=mybir.MatmulPerfMode.DoubleRow,
                )
            drain_pending(2)
            dst = Es[:, c * EXPW:(c + 1) * EXPW]
            if pat[c] == "A":
                nc.scalar.activation(
                    out=dst, in_=ps,
                    func=mybir.ActivationFunctionType.Exp,
                    scale=1.0 / 16.0,
                    accum_out=rs4[:, slot:slot + 1],
                )
                slot += 1
            else:
                nc.vector.tensor_scalar(
                    out=dst.bitcast(U16), in0=ps,
                    scalar1=SCH_A, scalar2=SCH_B,
                    op0=ALU.mult, op1=ALU.add,
                )
        # rowsum for V chunks: one DVE pass with fused accum; non-adjacent
        # V chunks are covered by a strided access pattern
        rsum = rsp.tile([P, 1], F32, tag="rsum", name=f"rsum{qi}")
        vcs = [c for c in range(NEXP) if pat[c] == "V"]
        if vcs:
            stride = (vcs[1] - vcs[0]) * EXPW if len(vcs) > 1 else EXPW
            src_ap = bass.AP(
                tensor=Es.tensor,
                offset=Es.offset + vcs[0] * EXPW,
                ap=[Es.ap[0], [stride, len(vcs)], [1, EXPW]],
            )
            junk = jkp.tile([P, S // 2], BF16, tag="jk", name=f"jk{qi}")
            nc.vector.tensor_scalar(
                out=junk[:, 0:len(vcs) * EXPW], in0=src_ap,
                scalar1=1.0, scalar2=0.0,
                op0=ALU.mult, op1=ALU.add,
                accum_out=rs4[:, slot:slot + 1],
            )
            slot += 1
        nc.vector.reduce_sum(
            out=rsum, in_=rs4[:, 0:slot], axis=mybir.AxisListType.X)
        recf = rsp.tile([P, 1], F32, tag="recf", name=f"recf{qi}")
        nc.vector.reciprocal(out=recf, in_=rsum)
        nc.gpsimd.tensor_scalar(
            out=lt_of[qi][:, 8:9], in0=recf,
            scalar1=1.0, scalar2=0.0,
            op0=ALU.mult, op1=ALU.add,
        )

    def emit_colsum(qi):
        Es = es_tiles.pop(qi)
        l = lt_of.pop(qi)

        def mk(m):
            def go():
                nc.tensor.matmul(
                    w_ps[m % 2],
                    l[:, 8 - m:16 - m],
                    Es[:, m * 512:(m + 1) * 512],
                    start=(qi == 0 and m < 2),
                    stop=(qi == QTILES - 1 and m >= 6),
                )
            return go
        for m in range(8):
            pending.append(mk(m))

    done = 0
    for qi in range(QTILES):
        emit_tile(qi)
        while done <= qi - COLSUM_LAG:
            emit_colsum(done)
            done += 1
    while done < QTILES:
        emit_colsum(done)
        done += 1
    drain_pending(len(pending))

    nc.vector.tensor_copy(out=w_sb[:, 0:512], in_=w_ps[0])
    nc.scalar.activation(out=w_sb[:, 512:1024], in_=w_ps[1],
                         func=mybir.ActivationFunctionType.Copy)
    nc.sync.dma_start(out=w_d[:, :], in_=w_sb)


_NC_CACHE = None


def _build_nc():
    global _NC_CACHE
    if _NC_CACHE is None:
        from contextlib import ExitStack

        nc = bacc.Bacc("TRN2", target_bir_lowering=False, debug=False)
        with tile.TileContext(nc) as tc, ExitStack() as ctx:
            _emit(ctx, tc)
        nc.compile()
        _NC_CACHE = nc
    return _NC_CACHE


def _in_maps(inputs):
    import ml_dtypes

    e4 = ml_dtypes.float8_e4m3
    x = np.asarray(inputs["x"], dtype=np.float32)
    Wq = np.asarray(inputs["Wq"], dtype=np.float32)
    Wk = np.asarray(inputs["Wk"], dtype=np.float32)
    bq = np.asarray(inputs["bq"], dtype=np.float32)
    bk = np.asarray(inputs["bk"], dtype=np.float32)
    maps = []
    for c in range(N_CORES):
        b, h = divmod(c, 2)
        q = x[b, h * HALF:(h + 1) * HALF] @ Wq + bq          # [HALF, E]
        k = x[b] @ Wk + bk                                   # [S, E]
        # [E, n] -> [128, 2, n] with middle dim = E-chunk (eo)
        qt8 = np.ascontiguousarray(
            q.T.reshape(2, P, HALF).transpose(1, 0, 2)).astype(e4)
        kt8 = np.ascontiguousarray(
            k.T.reshape(2, P, S).transpose(1, 0, 2)).astype(e4)
        maps.append({"qt8": qt8, "kt8": kt8})
    return maps


def _combine(results, inputs):
    x = np.asarray(inputs["x"], dtype=np.float64)
    Wv = np.asarray(inputs["Wv"], dtype=np.float64)
    bv = np.asarray(inputs["bv"], dtype=np.float64)
    out = np.empty((B, 1, E), dtype=np.float32)
    for b in range(B):
        wt = (results[2 * b]["w"].astype(np.float64)
              + results[2 * b + 1]["w"].astype(np.float64))
        w = (wt[:, 0:512] + wt[:, 512:1024]).reshape(S)
        u = w @ x[b]
        out[b, 0] = ((u / S) @ Wv + bv).astype(np.float32)
    return out


def kernel(**inputs):
    from concourse.bass_utils import run_bass_kernel_spmd

    nc = _build_nc()
    res = run_bass_kernel_spmd(nc, _in_maps(inputs), core_ids=list(range(N_CORES)))
    return _combine(res.results, inputs)


# revision 29
# speedup vs baseline: 1.0013x; 1.0013x over previous
"""Attention-pooling Trainium2 kernel, v5.

Per-core (8 cores = 4 batches x 2 query-row halves): [2048, 4096] score
block via fp8 DoubleRow matmuls (256-contraction in one pass, 0.5
cyc/row). exp splits between ACT (exact exp, bf16 out, fused rowsum
accumulator) and DVE (Schraudolph uint16 bit-trick writing bf16 bit
patterns; GPSIMD can read neither PSUM nor do free-axis reduces, so it
only issues DMAs). Rowsums for DVE chunks ride a 4x-mode bf16
tensor_scalar pass with fused accumulator. Colsums are M=8
sliding-window matmuls: lhsT = [128,16] zero tile with recb in col 8;
key-chunk m uses cols [8-m:16-m] so its output lands on psum partition
m, all 128 matmuls accumulating into one PSUM bank.

Host: Q/K projections + fp8 quantization in, (w @ x) @ Wv finish out.
"""

import numpy as np

import concourse.bass as bass  # noqa: F401
import concourse.mybir as mybir
import concourse.tile as tile
from concourse import bacc

B, S, E = 4, 4096, 256
HALF = S // 2
P = 128
N_CORES = 8
QTILES = HALF // P     # 16
F32 = mybir.dt.float32
FP8 = mybir.dt.float8e4
BF16 = mybir.dt.bfloat16
U16 = mybir.dt.uint16
ALU = mybir.AluOpType

EXPW = 1024            # psum chunk width
NEXP = S // EXPW       # 4 chunks per q-tile

LOG2E = float(np.log2(np.e))
SCH_A = 128.0 * LOG2E / 16.0          # Schraudolph scale (raw scores -> bf16 bits)
SCH_B = 16256.0 - 8.0                 # 127<<7, -8 = sawtooth calibration

# per-(tile, chunk) exp engine: A=ACT exact exp, V=DVE Schraudolph
SCHED = ["VVAA", "VAAA", "VVAA", "VAAA", "VVAA", "VAAA", "VVAA", "VAAA",
         "VVAA", "VAAA", "VVAA", "VAAA", "VAAA", "VVAA", "VAAA", "VAAA"]
COLSUM_LAG = 2  # tiles between exp emission and tile colsum emission
NLT = 6         # recb lhsT ring depth (must exceed COLSUM_LAG + 1)


def _runs(pat):
    """Contiguous 'V' chunk runs [(start, len)] for one tile pattern."""
    out = []
    c = 0
    while c < NEXP:
        if pat[c] == "A":
            c += 1
            continue
        c0 = c
        while c < NEXP and pat[c] != "A":
            c += 1
        out.append((c0, c - c0))
    return out


def _emit(ctx, tc):
    nc = tc.nc

    qt_d = nc.dram_tensor("qt8", [P, 2, HALF], FP8, kind="ExternalInput")
    kt_d = nc.dram_tensor("kt8", [P, 2, S], FP8, kind="ExternalInput")
    w_d = nc.dram_tensor("w", [8, 1024], F32, kind="ExternalOutput")

    const = ctx.enter_context(tc.tile_pool(name="const", bufs=1))
    epool = ctx.enter_context(tc.tile_pool(name="epool", bufs=5))
    rsp = ctx.enter_context(tc.tile_pool(name="rsp", bufs=4))
    jkp = ctx.enter_context(tc.tile_pool(name="jkp", bufs=2))
    pp = ctx.enter_context(tc.tile_pool(name="pp", bufs=3, space="PSUM"))
    wp = ctx.enter_context(tc.tile_pool(name="wp", bufs=1, space="PSUM"))

    qt_sb = const.tile([P, 2, HALF], FP8, name="qt_sb")
    kt_sb = const.tile([P, 2, S], FP8, name="kt_sb")
    lt = [const.tile([P, 16], BF16, name=f"lt{i}") for i in range(NLT)]
    for i in range(NLT):
        nc.vector.memset(lt[i], 0.0)
    w_sb = const.tile([8, 1024], F32, name="w_sb")

    # ---- input DMAs: tiny first-needed slices, then bulk on 3 queues
    nc.scalar.dma_start(out=kt_sb[:, :, 0:64], in_=kt_d[:, :, 0:64])
    nc.sync.dma_start(out=qt_sb[:, :, 0:128], in_=qt_d[:, :, 0:128])
    nc.gpsimd.dma_start(out=kt_sb[:, :, 64:512], in_=kt_d[:, :, 64:512])
    nc.scalar.dma_start(out=kt_sb[:, :, 512:1024], in_=kt_d[:, :, 512:1024])
    nc.sync.dma_start(out=kt_sb[:, :, 1024:1536], in_=kt_d[:, :, 1024:1536])
    nc.gpsimd.dma_start(out=kt_sb[:, :, 1536:2048], in_=kt_d[:, :, 1536:2048])
    nc.scalar.dma_start(out=kt_sb[:, :, 2048:2560], in_=kt_d[:, :, 2048:2560])
    nc.sync.dma_start(out=kt_sb[:, :, 2560:3072], in_=kt_d[:, :, 2560:3072])
    nc.gpsimd.dma_start(out=kt_sb[:, :, 3072:4096], in_=kt_d[:, :, 3072:4096])
    nc.scalar.dma_start(out=qt_sb[:, :, 128:1024], in_=qt_d[:, :, 128:1024])
    nc.sync.dma_start(out=qt_sb[:, :, 1024:2048], in_=qt_d[:, :, 1024:2048])

    w_ps = [wp.tile([8, 512], F32, name=f"w_ps{r}") for r in range(2)]
    es_tiles = {}
    lt_of = {}
    pending = []  # deferred colsum matmul thunks, drained ~1 per chunk

    def drain_pending(n):
        for _ in range(min(n, len(pending))):
            pending.pop(0)()

    def emit_tile(qi):
        Es = epool.tile([P, S], BF16, tag="E", name=f"E{qi}")
        es_tiles[qi] = Es
        lt_of[qi] = lt[qi % NLT]
        pat = SCHED[qi]
        rs4 = rsp.tile([P, 4], F32, tag="rs4", name=f"rs4_{qi}")
        slot = 0
        for c in range(NEXP):
            ps = pp.tile([P, EXPW], F32, tag="ps", name=f"ps{qi}_{c}")
            for h in range(2):
                t0 = c * EXPW + h * 512
                nc.tensor.matmul(
                    ps[:, h * 512:(h + 1) * 512],
                    qt_sb[:, :, qi * P:(qi + 1) * P],
                    kt_sb[:, :, t0:t0 + 512],
                    start=True, stop=True,
                    perf_mode=mybir.MatmulPerfMode.DoubleRow,
                )
            drain_pending(2)
            dst = Es[:, c * EXPW:(c + 1) * EXPW]
            if pat[c] == "A":
                nc.scalar.activation(
                    out=dst, in_=ps,
                    func=mybir.ActivationFunctionType.Exp,
                    scale=1.0 / 16.0,
                    accum_out=rs4[:, slot:slot + 1],
                )
                slot += 1
            else:
                nc.vector.tensor_scalar(
                    out=dst.bitcast(U16), in0=ps,
                    scalar1=SCH_A, scalar2=SCH_B,
                    op0=ALU.mult, op1=ALU.add,
                )
        # rowsum for V chunks: one DVE pass with fused accum; non-adjacent
        # V chunks are covered by a strided access pattern
        rsum = rsp.tile([P, 1], F32, tag="rsum", name=f"rsum{qi}")
        vcs = [c for c in range(NEXP) if pat[c] == "V"]
        if vcs:
            stride = (vcs[1] - vcs[0]) * EXPW if len(vcs) > 1 else EXPW
            src_ap = bass.AP(
                tensor=Es.tensor,
                offset=Es.offset + vcs[0] * EXPW,
                ap=[Es.ap[0], [stride, len(vcs)], [1, EXPW]],
            )
            junk = jkp.tile([P, S // 2], BF16, tag="jk", name=f"jk{qi}")
            nc.vector.tensor_scalar(
                out=junk[:, 0:len(vcs) * EXPW], in0=src_ap,
                scalar1=1.0, scalar2=0.0,
                op0=ALU.mult, op1=ALU.add,
                accum_out=rs4[:, slot:slot + 1],
            )
            slot += 1
        nc.vector.reduce_sum(
            out=rsum, in_=rs4[:, 0:slot], axis=mybir.AxisListType.X)
        recf = rsp.tile([P, 1], F32, tag="recf", name=f"recf{qi}")
        nc.vector.reciprocal(out=recf, in_=rsum)
        nc.gpsimd.tensor_scalar(
            out=lt_of[qi][:, 8:9], in0=recf,
            scalar1=1.0, scalar2=0.0,
            op0=ALU.mult, op1=ALU.add,
        )

    def emit_colsum(qi):
        Es = es_tiles.pop(qi)
        l = lt_of.pop(qi)

        def mk(m):
            def go():
                nc.tensor.matmul(
                    w_ps[m % 2],
                    l[:, 8 - m:16 - m],
                    Es[:, m * 512:(m + 1) * 512],
                    start=(qi == 0 and m < 2),
                    stop=(qi == QTILES - 1 and m >= 6),
                )
            return go
        for m in range(8):
            pending.append(mk(m))

    done = 0
    for qi in range(QTILES):
        emit_tile(qi)
        while done <= qi - COLSUM_LAG:
            emit_colsum(done)
            done += 1
    while done < QTILES:
        emit_colsum(done)
        done += 1
    drain_pending(len(pending))

    nc.vector.tensor_copy(out=w_sb[:, 0:512], in_=w_ps[0])
    nc.scalar.activation(out=w_sb[:, 512:1024], in_=w_ps[1],
                         func=mybir.ActivationFunctionType.Copy)
    nc.sync.dma_start(out=w_d[:, :], in_=w_sb)


_NC_CACHE = None


def _build_nc():
    global _NC_CACHE
    if _NC_CACHE is None:
        from contextlib import ExitStack

        nc = bacc.Bacc("TRN2", target_bir_lowering=False, debug=False)
        with tile.TileContext(nc) as tc, ExitStack() as ctx:
            _emit(ctx, tc)
        nc.compile()
        _NC_CACHE = nc
    return _NC_CACHE


def _in_maps(inputs):
    import ml_dtypes

    e4 = ml_dtypes.float8_e4m3
    x = np.asarray(inputs["x"], dtype=np.float32)
    Wq = np.asarray(inputs["Wq"], dtype=np.float32)
    Wk = np.asarray(inputs["Wk"], dtype=np.float32)
    bq = np.asarray(inputs["bq"], dtype=np.float32)
    bk = np.asarray(inputs["bk"], dtype=np.float32)
    maps = []
    for c in range(N_CORES):
        b, h = divmod(c, 2)
        q = x[b, h * HALF:(h + 1) * HALF] @ Wq + bq          # [HALF, E]
        k = x[b] @ Wk + bk                                   # [S, E]
        # [E, n] -> [128, 2, n] with middle dim = E-chunk (eo)
        qt8 = np.ascontiguousarray(
            q.T.reshape(2, P, HALF).transpose(1, 0, 2)).astype(e4)
        kt8 = np.ascontiguousarray(
            k.T.reshape(2, P, S).transpose(1, 0, 2)).astype(e4)
        maps.append({"qt8": qt8, "kt8": kt8})
    return maps


def _combine(results, inputs):
    x = np.asarray(inputs["x"], dtype=np.float64)
    Wv = np.asarray(inputs["Wv"], dtype=np.float64)
    bv = np.asarray(inputs["bv"], dtype=np.float64)
    out = np.empty((B, 1, E), dtype=np.float32)
    for b in range(B):
        wt = (results[2 * b]["w"].astype(np.float64)
              + results[2 * b + 1]["w"].astype(np.float64))
        w = (wt[:, 0:512] + wt[:, 512:1024]).reshape(S)
        u = w @ x[b]
        out[b, 0] = ((u / S) @ Wv + bv).astype(np.float32)
    return out


def kernel(**inputs):
    from concourse.bass_utils import run_bass_kernel_spmd

    nc = _build_nc()
    res = run_bass_kernel_spmd(nc, _in_maps(inputs), core_ids=list(range(N_CORES)))
    return _combine(res.results, inputs)


# revision 31
# speedup vs baseline: 1.0744x; 1.0730x over previous
"""Attention-pooling Trainium2 kernel, v5.

Per-core (8 cores = 4 batches x 2 query-row halves): [2048, 4096] score
block via fp8 DoubleRow matmuls (256-contraction in one pass, 0.5
cyc/row). exp splits between ACT (exact exp, bf16 out, fused rowsum
accumulator) and DVE (Schraudolph uint16 bit-trick writing bf16 bit
patterns; GPSIMD can read neither PSUM nor do free-axis reduces, so it
only issues DMAs). Rowsums for DVE chunks ride a 4x-mode bf16
tensor_scalar pass with fused accumulator. Colsums are M=8
sliding-window matmuls: lhsT = [128,16] zero tile with recb in col 8;
key-chunk m uses cols [8-m:16-m] so its output lands on psum partition
m, all 128 matmuls accumulating into one PSUM bank.

Host: Q/K projections + fp8 quantization in, (w @ x) @ Wv finish out.
"""

import numpy as np

import concourse.bass as bass  # noqa: F401
import concourse.mybir as mybir
import concourse.tile as tile
from concourse import bacc

B, S, E = 4, 4096, 256
HALF = S // 2
P = 128
N_CORES = 8
QTILES = HALF // P     # 16
F32 = mybir.dt.float32
FP8 = mybir.dt.float8e4
BF16 = mybir.dt.bfloat16
U16 = mybir.dt.uint16
U8 = mybir.dt.uint8
ALU = mybir.AluOpType

EXPW = 1024            # psum chunk width
NEXP = S // EXPW       # 4 chunks per q-tile

LOG2E = float(np.log2(np.e))
CSHIFT = 2.0                          # exp(score/16 - CSHIFT): keeps E in fp8 range
SCH_A = 8.0 * LOG2E / 16.0            # Schraudolph scale (raw scores -> e4m3 bits)
SCH_B = 56.0 - 8.0 * CSHIFT * LOG2E - 0.5   # bias, -0.5 = sawtooth calibration
RECB_SCALE = 1024.0                   # recb prescale (undone on host)

# per-(tile, chunk) exp engine: A=ACT exact exp, V=DVE Schraudolph
SCHED = ["VVAA", "VAAA", "VVAA", "VAAA", "VVAA", "VAAA", "VVAA", "VAAA",
         "VVAA", "VAAA", "VVAA", "VAAA", "VAAA", "VVAA", "VAAA", "VAAA"]
COLSUM_LAG = 2  # tiles between exp emission and tile colsum emission
NLT = 6         # recb lhsT ring depth (must exceed COLSUM_LAG + 1)


def _runs(pat):
    """Contiguous 'V' chunk runs [(start, len)] for one tile pattern."""
    out = []
    c = 0
    while c < NEXP:
        if pat[c] == "A":
            c += 1
            continue
        c0 = c
        while c < NEXP and pat[c] != "A":
            c += 1
        out.append((c0, c - c0))
    return out


def _emit(ctx, tc):
    nc = tc.nc

    qt_d = nc.dram_tensor("qt8", [P, 2, HALF], FP8, kind="ExternalInput")
    kt_d = nc.dram_tensor("kt8", [P, 2, S], FP8, kind="ExternalInput")
    w_d = nc.dram_tensor("w", [8, 1024], F32, kind="ExternalOutput")

    const = ctx.enter_context(tc.tile_pool(name="const", bufs=1))
    epool = ctx.enter_context(tc.tile_pool(name="epool", bufs=5))
    rsp = ctx.enter_context(tc.tile_pool(name="rsp", bufs=4))
    jkp = ctx.enter_context(tc.tile_pool(name="jkp", bufs=2))
    pp = ctx.enter_context(tc.tile_pool(name="pp", bufs=3, space="PSUM"))
    wp = ctx.enter_context(tc.tile_pool(name="wp", bufs=1, space="PSUM"))

    qt_sb = const.tile([P, 2, HALF], FP8, name="qt_sb")
    kt_sb = const.tile([P, 2, S], FP8, name="kt_sb")
    bias_t = const.tile([P, 1], F32, name="bias_t")
    nc.vector.memset(bias_t, -CSHIFT)
    lt = [const.tile([P, 2, 16], FP8, name=f"lt{i}") for i in range(NLT)]
    for i in range(NLT):
        nc.vector.memset(lt[i], 0.0)
    w_sb = const.tile([8, 1024], F32, name="w_sb")

    # ---- input DMAs: tiny first-needed slices, then bulk on 3 queues
    nc.scalar.dma_start(out=kt_sb[:, :, 0:64], in_=kt_d[:, :, 0:64])
    nc.sync.dma_start(out=qt_sb[:, :, 0:128], in_=qt_d[:, :, 0:128])
    nc.gpsimd.dma_start(out=kt_sb[:, :, 64:512], in_=kt_d[:, :, 64:512])
    nc.scalar.dma_start(out=kt_sb[:, :, 512:1024], in_=kt_d[:, :, 512:1024])
    nc.sync.dma_start(out=kt_sb[:, :, 1024:1536], in_=kt_d[:, :, 1024:1536])
    nc.gpsimd.dma_start(out=kt_sb[:, :, 1536:2048], in_=kt_d[:, :, 1536:2048])
    nc.scalar.dma_start(out=kt_sb[:, :, 2048:2560], in_=kt_d[:, :, 2048:2560])
    nc.sync.dma_start(out=kt_sb[:, :, 2560:3072], in_=kt_d[:, :, 2560:3072])
    nc.gpsimd.dma_start(out=kt_sb[:, :, 3072:4096], in_=kt_d[:, :, 3072:4096])
    nc.scalar.dma_start(out=qt_sb[:, :, 128:1024], in_=qt_d[:, :, 128:1024])
    nc.sync.dma_start(out=qt_sb[:, :, 1024:2048], in_=qt_d[:, :, 1024:2048])

    w_ps = [wp.tile([8, 512], F32, name=f"w_ps{r}") for r in range(2)]
    es_tiles = {}
    lt_of = {}
    pending = []  # deferred colsum matmul thunks, drained ~1 per chunk

    def drain_pending(n):
        for _ in range(min(n, len(pending))):
            pending.pop(0)()

    def emit_tile(qi):
        j = qi % 2
        pair = qi // 2
        if j == 0:
            es_tiles[pair] = epool.tile([P, 2, S], FP8, tag="E", name=f"E{pair}")
            lt_of[pair] = lt[pair % NLT]
        Es = es_tiles[pair]
        pat = SCHED[qi]
        rs4 = rsp.tile([P, 4], F32, tag="rs4", name=f"rs4_{qi}")
        slot = 0
        for c in range(NEXP):
            ps = pp.tile([P, EXPW], F32, tag="ps", name=f"ps{qi}_{c}")
            for h in range(2):
                t0 = c * EXPW + h * 512
                nc.tensor.matmul(
                    ps[:, h * 512:(h + 1) * 512],
                    qt_sb[:, :, qi * P:(qi + 1) * P],
                    kt_sb[:, :, t0:t0 + 512],
                    start=True, stop=True,
                    perf_mode=mybir.MatmulPerfMode.DoubleRow,
                )
            drain_pending(2)
            dst = Es[:, j, c * EXPW:(c + 1) * EXPW]
            if pat[c] == "A":
                nc.scalar.activation(
                    out=dst, in_=ps,
                    func=mybir.ActivationFunctionType.Exp,
                    scale=1.0 / 16.0, bias=bias_t,
                    accum_out=rs4[:, slot:slot + 1],
                )
                slot += 1
            else:
                nc.vector.tensor_scalar(
                    out=dst.bitcast(U8), in0=ps,
                    scalar1=SCH_A, scalar2=SCH_B,
                    op0=ALU.mult, op1=ALU.add,
                )
        # rowsum for V chunks: one DVE pass with fused accum; non-adjacent
        # V chunks are covered by a strided access pattern
        rsum = rsp.tile([P, 1], F32, tag="rsum", name=f"rsum{qi}")
        vcs = [c for c in range(NEXP) if pat[c] == "V"]
        if vcs:
            stride = (vcs[1] - vcs[0]) * EXPW if len(vcs) > 1 else EXPW
            src_ap = bass.AP(
                tensor=Es.tensor,
                offset=Es.offset + j * S + vcs[0] * EXPW,
                ap=[Es.ap[0], [stride, len(vcs)], [1, EXPW]],
            )
            junk = jkp.tile([P, S // 2], FP8, tag="jk", name=f"jk{qi}")
            nc.vector.tensor_scalar(
                out=junk[:, 0:len(vcs) * EXPW], in0=src_ap,
                scalar1=1.0, scalar2=0.0,
                op0=ALU.mult, op1=ALU.add,
                accum_out=rs4[:, slot:slot + 1],
            )
            slot += 1
        nc.vector.reduce_sum(
            out=rsum, in_=rs4[:, 0:slot], axis=mybir.AxisListType.X)
        recf = rsp.tile([P, 1], F32, tag="recf", name=f"recf{qi}")
        nc.vector.reciprocal(out=recf, in_=rsum)
        nc.gpsimd.tensor_scalar(
            out=lt_of[pair][:, j, 8:9], in0=recf,
            scalar1=RECB_SCALE, scalar2=0.0,
            op0=ALU.mult, op1=ALU.add,
        )

    def emit_colsum(pair):
        Es = es_tiles.pop(pair)
        l = lt_of.pop(pair)
        NP = QTILES // 2

        def mk(m):
            def go():
                nc.tensor.matmul(
                    w_ps[m % 2],
                    l[:, :, 8 - m:16 - m],
                    Es[:, :, m * 512:(m + 1) * 512],
                    start=(pair == 0 and m < 2),
                    stop=(pair == NP - 1 and m >= 6),
                    perf_mode=mybir.MatmulPerfMode.DoubleRow,
                )
            return go
        for m in range(8):
            pending.append(mk(m))

    done = 0
    for qi in range(QTILES):
        emit_tile(qi)
        ready = (qi - COLSUM_LAG + 1) // 2
        while done < ready:
            emit_colsum(done)
            done += 1
    while done < QTILES // 2:
        emit_colsum(done)
        done += 1
    drain_pending(len(pending))

    nc.vector.tensor_copy(out=w_sb[:, 0:512], in_=w_ps[0])
    nc.scalar.activation(out=w_sb[:, 512:1024], in_=w_ps[1],
                         func=mybir.ActivationFunctionType.Copy)
    nc.sync.dma_start(out=w_d[:, :], in_=w_sb)


_NC_CACHE = None


def _build_nc():
    global _NC_CACHE
    if _NC_CACHE is None:
        from contextlib import ExitStack

        nc = bacc.Bacc("TRN2", target_bir_lowering=False, debug=False)
        with tile.TileContext(nc) as tc, ExitStack() as ctx:
            _emit(ctx, tc)
        nc.compile()
        _NC_CACHE = nc
    return _NC_CACHE


def _in_maps(inputs):
    import ml_dtypes

    e4 = ml_dtypes.float8_e4m3
    x = np.asarray(inputs["x"], dtype=np.float32)
    Wq = np.asarray(inputs["Wq"], dtype=np.float32)
    Wk = np.asarray(inputs["Wk"], dtype=np.float32)
    bq = np.asarray(inputs["bq"], dtype=np.float32)
    bk = np.asarray(inputs["bk"], dtype=np.float32)
    maps = []
    for c in range(N_CORES):
        b, h = divmod(c, 2)
        q = x[b, h * HALF:(h + 1) * HALF] @ Wq + bq          # [HALF, E]
        k = x[b] @ Wk + bk                                   # [S, E]
        # [E, n] -> [128, 2, n] with middle dim = E-chunk (eo)
        qt8 = np.ascontiguousarray(
            q.T.reshape(2, P, HALF).transpose(1, 0, 2)).astype(e4)
        kt8 = np.ascontiguousarray(
            k.T.reshape(2, P, S).transpose(1, 0, 2)).astype(e4)
        maps.append({"qt8": qt8, "kt8": kt8})
    return maps


def _combine(results, inputs):
    x = np.asarray(inputs["x"], dtype=np.float64)
    Wv = np.asarray(inputs["Wv"], dtype=np.float64)
    bv = np.asarray(inputs["bv"], dtype=np.float64)
    out = np.empty((B, 1, E), dtype=np.float32)
    for b in range(B):
        wt = (results[2 * b]["w"].astype(np.float64)
              + results[2 * b + 1]["w"].astype(np.float64))
        w = (wt[:, 0:512] + wt[:, 512:1024]).reshape(S) / RECB_SCALE
        u = w @ x[b]
        out[b, 0] = ((u / S) @ Wv + bv).astype(np.float32)
    return out


def kernel(**inputs):
    from concourse.bass_utils import run_bass_kernel_spmd

    nc = _build_nc()
    res = run_bass_kernel_spmd(nc, _in_maps(inputs), core_ids=list(range(N_CORES)))
    return _combine(res.results, inputs)
